# revision 4
# baseline (speedup 1.0000x reference)
"""Involution kernel for Trainium2, 8 NeuronCores (v3).

Sharding: data-parallel over (batch=4) x (H halves=2); 28 output rows
per core, 3-row halo, 60-wide padded raster (W=60: all tap shifts stay
8B-aligned since 60 % 4 == 0; wrapped +-3-col tap reads land in the
2-col side pads of adjacent rows, which are zero == conv padding).

Channels are cg-major on partitions (partition p of tile t <-> channel
(p%16)*16 + p//16 + 8t), so the 16 groups sit on p%16 and one 8x
row-replicated span matmul yields per-pixel weights wm_k [128, span]
valid for BOTH channel tiles.

Per tap k:
  PE:   wm_k = w2rep_k^T @ r   (2-bank half-tap PSUM tiles, dbl-buffered)
  Act:  wmb_k = wm_k + b2_k -> fp16 SBUF, compact 28x56 (b2 via bias)
  DVE:  prod = wmb_k * x_shift  [128, 2, 28, 56] both tiles, 2x mode
  PE:   acc_t0 += prod[:,0]     (identity matmul into 4-bank PSUM acc)
  SWDGE/DVE: acc_t1 += prod[:,1] (gpsimd accumulate-DMA or DVE add)
Outputs: acc_t0 PSUM -> ScalarE copy f32 -> DMA; acc_t1 fp16 -> gpsimd
cast-DMA to f32 DRAM.
"""

import os
import sys
import numpy as np

for _p in ("/opt/trn_rl_repo",):
    if _p not in sys.path:
        sys.path.insert(0, _p)

import concourse.bass as bass
import concourse.tile as tile
from concourse import mybir
from concourse.bass_utils import run_bass_kernel_spmd
import bass_rust

F32 = mybir.dt.float32
F16 = mybir.dt.float16

N_CORES = 8
C = 256
RED = 128
K = 7
K2 = 49
GC = 16
HW = 56
W60 = 60
ROWS = 34
NPIX = ROWS * W60        # 2040
PAD = 8
XLEN = NPIX + 2 * PAD    # 2056
Q0 = 3 * W60             # 180
SPAN = 28 * W60          # 1680 raster cols covering own rows
CH = SPAN // 4           # 420 raster cols per PSUM bank
NOUT = 28 * HW           # 1568 compact output pixels
CHO = NOUT // 4          # 392 compact cols per PSUM bank
OUT0 = PAD + Q0 + 2      # flat index of first own output pixel

def _tap_delta(k):
    di, dj = k // K, k % K
    return (di - 3) * W60 + (dj - 3)


# process taps grouped by which shifted x copy they read: j==0 taps first
# (xs0 is the first copy DMA'd in), then j==1, 2, 3 as the copies stream
# in; within a group, halo-free taps (di==3) first so the very first taps
# don't wait on the halo-row loads
TAPORD = sorted(range(K2), key=lambda k: (_tap_delta(k) % 4, k // K != 3, k))
# positions (in TAPORD) whose t1-accumulate runs on DVE instead of the
# gpsimd accum-DMA; the tail block keeps its latency off the SWDGE path
DVE_ACC_POS = frozenset({45, 46, 47, 48})
W2CHUNKS = 4


def _split_multi_waits(nc, maxw=1):
    """This walrus build caps sync-wait commands per instruction; move
    excess waits onto same-engine nops inserted immediately before."""
    ctr = 0
    for fn in nc.m.functions:
        for bb in fn.blocks:
            insts = bb.instructions  # live list
            i = 0
            while i < len(insts):
                ins = insts[i]
                si = ins.sync_info
                waits = list(si.on_wait) if si is not None else []
                if len(waits) > maxw:
                    excess, keep = waits[:-maxw], waits[-maxw:]
                    for j in range(0, len(excess), maxw):
                        ctr += 1
                        nop = mybir.InstNoOp(
                            name=f"waitsplit-{ctr}",
                            engine=ins.engine,
                            bass_nofuse=True,
                            sync_info=mybir.SyncInfo(
                                on_wait=excess[j:j + maxw], on_update=[]
                            ),
                        )
                        insts.insert(i, nop)
                        i += 1
                    ins.sync_info = bass_rust.SyncInfo(
                        on_wait=keep, on_update=list(si.on_update)
                    )
                i += 1


def build_program():
    nc = bass.Bass("TRN2", target_bir_lowering=False, num_devices=N_CORES)

    xs_d = [nc.dram_tensor(f"xs{j}", [128, 2 * XLEN], F16, kind="ExternalInput")
            for j in range(4)]
    # packed small weights in one fp16 DMA (w1 tiles | identity | b1 | b2)
    # -- each HWDGE DMA costs ~625ns serialized at startup
    p16_d = nc.dram_tensor("p16", [128, 3 * RED + 1 + K2], F16,
                           kind="ExternalInput")
    # w2 blocks stored in TAPORD processing order, split into chunks
    w2_d = nc.dram_tensor("w2L", [RED, K2 * 128], F16, kind="ExternalInput")
    y_d = nc.dram_tensor("y", [128, 2, 28, HW], F32, kind="ExternalOutput")
    csz = [(K2 + W2CHUNKS - 1) // W2CHUNKS] * W2CHUNKS
    csz[-1] = K2 - sum(csz[:-1])

    with tile.TileContext(nc) as tc:
        with (
            tc.tile_pool(name="sb", bufs=1) as sb,
            tc.tile_pool(name="dbl", bufs=6) as dbl,
            tc.tile_pool(name="ps", bufs=2, space="PSUM") as ps,
            tc.tile_pool(name="psacc", bufs=1, space="PSUM") as psacc,
        ):
            # load order: packed small weights, xs0 own rows (gates phase
            # A), first tap's w2 block, halos, then the rest streaming in
            p16 = sb.tile([128, 3 * RED + 1 + K2], F16, tag="p16")
            nc.sync.dma_start(out=p16[:], in_=p16_d[:, :])
            w1sb = [p16[:, 0:RED], p16[:, RED:2 * RED]]
            idsb = p16[:, 2 * RED:3 * RED]
            b1sb = p16[:, 3 * RED:3 * RED + 1]
            b2sb = p16[:, 3 * RED + 1:]

            xs = []
            for j in range(4):
                t = sb.tile([128, 2 * XLEN], F16, tag=f"xs{j}")
                xs.append(t)
            xs0v = xs[0][:].rearrange("p (t l) -> p t l", t=2)
            xs0dv = xs_d[0][:, :].rearrange("p (t l) -> p t l", t=2)
            for h in range(2):
                o = PAD + Q0 + h * (SPAN // 2)
                nc.sync.dma_start(
                    out=xs0v[:, :, o:o + SPAN // 2],
                    in_=xs0dv[:, :, o:o + SPAN // 2],
                )
            w2sb = sb.tile([RED, K2 * 128], F16, tag="w2")
            nc.sync.dma_start(out=w2sb[:, 0:128], in_=w2_d[:, 0:128])
            nc.sync.dma_start(
                out=xs0v[:, :, 0:PAD + Q0],
                in_=xs0dv[:, :, 0:PAD + Q0],
            )
            nc.sync.dma_start(
                out=xs0v[:, :, PAD + Q0 + SPAN:XLEN],
                in_=xs0dv[:, :, PAD + Q0 + SPAN:XLEN],
            )

            co = 1
            for i, csz_i in enumerate(csz):
                csz_i = min(csz_i, K2 - co)
                nc.sync.dma_start(
                    out=w2sb[:, co * 128:(co + csz_i) * 128],
                    in_=w2_d[:, co * 128:(co + csz_i) * 128],
                )
                co += csz_i
                if i == 0:
                    nc.sync.dma_start(out=xs[1][:], in_=xs_d[1][:, :])
                elif i == 1:
                    nc.sync.dma_start(out=xs[2][:], in_=xs_d[2][:, :])
                elif i == 2:
                    nc.sync.dma_start(out=xs[3][:], in_=xs_d[3][:, :])

            # Phase A: r = relu(w1' @ x + b1')  [128, SPAN] fp16, 60-raster
            # (per-chunk relu so the first span matmuls release early)
            r_sb = sb.tile([RED, SPAN], F16, tag="r")
            for h in range(2):
                rps = ps.tile([128, 2, 512], F32, tag="wmps")
                for cc in range(2):
                    c4 = h * 2 + cc
                    for ci in range(2):
                        nc.tensor.matmul(
                            rps[:, cc, 0:CH],
                            w1sb[ci],
                            xs[0][:, ci * XLEN + PAD + Q0 + c4 * CH:
                                  ci * XLEN + PAD + Q0 + (c4 + 1) * CH],
                            start=(ci == 0),
                            stop=(ci == 1),
                        )
                    nc.scalar.activation(
                        out=r_sb[:, c4 * CH:(c4 + 1) * CH],
                        in_=rps[:, cc, 0:CH],
                        func=mybir.ActivationFunctionType.Relu,
                        bias=b1sb,
                        scale=1.0,
                    )

            # Phase B
            acc_ps = psacc.tile([128, 4, 512], F32, tag="acc")
            # 4 independent t1 partial accumulators -> 4 parallel RMW chains
            NACC = 4
            acc1s = []
            for i in range(NACC):
                a1t = sb.tile([128, NOUT], F16, tag=f"acc1_{i}")
                acc1s.append(a1t)
            acc1_first = [True] * NACC
            for pos, k in enumerate(TAPORD):
                delta = _tap_delta(k)
                j = delta % 4
                b0 = OUT0 + delta - j

                prod = dbl.tile([128, 2, NOUT], F16, tag="prod")
                prod_v = prod[:].rearrange("p t (hr c) -> p t hr c", c=HW)
                wmb = dbl.tile([128, NOUT], F16, tag="wmb")
                wmb_v = wmb[:].rearrange("p (h b r c) -> p h b r c",
                                         h=2, b=2, r=7, c=HW)
                wmb_m = wmb[:].rearrange("p (h rr c) -> p h rr c",
                                         h=2, rr=14, c=HW)
                xin = (xs[j][:].rearrange("p (t l) -> p t l", t=2)
                       [:, :, b0:b0 + 28 * W60]
                       .rearrange("p t (h r c) -> p t h r c", h=2, c=W60)
                       [:, :, :, :, 0:HW])

                # both span halves first, then both evacs(+b2): ScalarE
                # runs back-to-back instead of idling on span h1
                wmps_h = []
                for h in range(2):
                    wmps = ps.tile([128, 2, 512], F32, tag="wmps")
                    for hh in range(2):
                        c4 = h * 2 + hh
                        nc.tensor.matmul(
                            wmps[:, hh, 0:CH],
                            w2sb[:, pos * 128:(pos + 1) * 128],
                            r_sb[:, c4 * CH:(c4 + 1) * CH],
                            start=True,
                            stop=True,
                        )
                    wmps_h.append(wmps)
                for h in range(2):
                    src = wmps_h[h][:, :, 0:CH].rearrange(
                        "p b (r c) -> p b r c", c=W60)[:, :, :, 2:2 + HW]
                    nc.scalar.activation(
                        out=wmb_v[:, h],
                        in_=src,
                        func=mybir.ActivationFunctionType.Identity,
                        bias=b2sb[:, pos:pos + 1],
                        scale=1.0,
                    )

                # both-tile fused multiply on DVE (2x mode); the first taps
                # run as half multiplies so the pipeline fills faster
                wmb_b = (wmb[:].rearrange("p (r c) -> p r c", c=HW)
                         .unsqueeze(1).broadcast_to((128, 2, 28, HW)))
                xin_f = xin.rearrange("p t h r c -> p t (h r) c")
                if pos < 4:
                    for h in range(2):
                        nc.vector.tensor_tensor(
                            out=prod_v[:, :, h * 14:(h + 1) * 14],
                            in0=wmb_b[:, :, h * 14:(h + 1) * 14],
                            in1=xin_f[:, :, h * 14:(h + 1) * 14],
                            op=mybir.AluOpType.mult,
                        )
                else:
                    nc.vector.tensor_tensor(
                        out=prod_v, in0=wmb_b, in1=xin_f,
                        op=mybir.AluOpType.mult,
                    )

                # t0 accumulate on PE (identity matmul into PSUM)
                for c4 in range(4):
                    nc.tensor.matmul(
                        acc_ps[:, c4, 0:CHO],
                        idsb,
                        prod[:, 0, c4 * CHO:(c4 + 1) * CHO],
                        start=(pos == 0),
                        stop=(pos == K2 - 1),
                        skip_group_check=True,
                    )

                # t1 accumulate: gpsimd accum-DMA or DVE add; partials are
                # block-assigned (2-way interleaved per half) so early
                # partials can merge while later taps still run
                if pos >= 45:
                    ai = 0  # tail taps: short DVE chain on the idle partial 0
                else:
                    ai = (2 * ((2 * pos) // K2)) + (pos % 2)
                a1 = acc1s[ai]
                if acc1_first[ai]:
                    acc1_first[ai] = False
                    nc.sync.dma_start(out=a1[:], in_=prod[:, 1, :])
                elif pos in DVE_ACC_POS:
                    nc.vector.tensor_tensor(
                        out=a1[:], in0=prod[:, 1, :], in1=a1[:],
                        op=mybir.AluOpType.add,
                    )
                else:
                    nc.gpsimd.dma_start(
                        out=a1[:], in_=prod[:, 1, :],
                        accum_op=mybir.AluOpType.add,
                    )
                if pos == 30:
                    # partials 0/1 got their last tap at pos 24; fold 1 into 0
                    nc.vector.tensor_tensor(
                        out=acc1s[0][:], in0=acc1s[0][:], in1=acc1s[1][:],
                        op=mybir.AluOpType.add,
                    )
                elif pos == 47:
                    # a2/a3 got their last SWDGE tap at pos 44; pre-merge
                    nc.vector.tensor_tensor(
                        out=acc1s[2][:], in0=acc1s[2][:], in1=acc1s[3][:],
                        op=mybir.AluOpType.add,
                    )

            # Phase C: final partial merge and outputs, halved so the DMAs
            # overlap the merges/evacs of the other half
            y0 = dbl.tile([128, NOUT], F32, tag="y0")
            HF = NOUT // 2
            for h in range(2):
                nc.vector.tensor_tensor(
                    out=acc1s[0][:, h * HF:(h + 1) * HF],
                    in0=acc1s[0][:, h * HF:(h + 1) * HF],
                    in1=acc1s[2][:, h * HF:(h + 1) * HF],
                    op=mybir.AluOpType.add,
                )
                nc.gpsimd.dma_start(
                    out=y_d[:, 1, h * 14:(h + 1) * 14],
                    in_=acc1s[0][:, h * HF:(h + 1) * HF]
                    .rearrange("p (r c) -> p r c", c=HW),
                )
                nc.scalar.activation(
                    out=y0[:, h * HF:(h + 1) * HF],
                    in_=acc_ps[:, 2 * h:2 * (h + 1), 0:CHO],
                    func=mybir.ActivationFunctionType.Copy,
                    bias=0.0,
                    scale=1.0,
                )
                nc.sync.dma_start(
                    out=y_d[:, 0, h * 14:(h + 1) * 14],
                    in_=y0[:, h * HF:(h + 1) * HF]
                    .rearrange("p (r c) -> p r c", c=HW),
                )
    _split_multi_waits(nc)
    return nc


_PROGRAM = None
LAST_RESULT = None


def _perm_channels(t):
    p = np.arange(128)
    return (p % 16) * GC + (p // 16) + 8 * t


def kernel(x, w1, b1, gamma, beta, run_mean, run_var, w2, b2):
    global _PROGRAM, LAST_RESULT
    x = np.asarray(x, dtype=np.float32)
    w1 = np.asarray(w1, dtype=np.float32)
    b1 = np.asarray(b1, dtype=np.float32)
    gamma = np.asarray(gamma, dtype=np.float32)
    beta = np.asarray(beta, dtype=np.float32)
    run_mean = np.asarray(run_mean, dtype=np.float32)
    run_var = np.asarray(run_var, dtype=np.float32)
    w2 = np.asarray(w2, dtype=np.float32)
    b2 = np.asarray(b2, dtype=np.float32)

    B = x.shape[0]
    s = gamma / np.sqrt(run_var + 1e-5)
    w1p = w1 * s[:, None]
    b1p = (b1 - run_mean) * s + beta

    perm = [_perm_channels(0), _perm_channels(1)]

    p16 = np.empty((128, 3 * RED + 1 + K2), dtype=np.float16)
    for t in range(2):
        p16[:, t * RED:(t + 1) * RED] = w1p[:, perm[t]].T
    p16[:, 2 * RED:3 * RED] = np.eye(128, dtype=np.float16)
    p16[:, 3 * RED] = b1p

    p = np.arange(128)
    w2L = np.empty((RED, K2 * 128), dtype=np.float16)
    for pos, k in enumerate(TAPORD):
        rows = (p % 16) * K2 + k
        w2L[:, pos * 128:(pos + 1) * 128] = w2[rows, :].T
        p16[:, 3 * RED + 1 + pos] = b2[rows]

    xpad = np.zeros((B, C, 62, W60), dtype=np.float32)
    xpad[:, :, 3:3 + HW, 2:2 + HW] = x

    in_maps = []
    for core in range(N_CORES):
        b, half = core // 2, core % 2
        sh = xpad[b, :, half * 28: half * 28 + ROWS, :].reshape(C, NPIX)
        xsl = np.zeros((128, 2, XLEN), dtype=np.float16)
        for t in range(2):
            xsl[:, t, PAD:PAD + NPIX] = sh[perm[t], :]
        xflat = xsl.reshape(128, 2 * XLEN)
        m = {"p16": p16, "w2L": w2L}
        for j in range(4):
            xj = np.zeros_like(xflat)
            if j == 0:
                xj[:] = xflat
            else:
                xj[:, :-j] = xflat[:, j:]
            m[f"xs{j}"] = xj
        in_maps.append(m)

    if _PROGRAM is None:
        _PROGRAM = build_program()
    trace = os.environ.get("INVOL_TRACE") == "1"
    res = run_bass_kernel_spmd(
        _PROGRAM, in_maps, list(range(N_CORES)),
        trace=trace,
        tmpdir=os.environ.get("INVOL_TRACE_DIR") or None,
    )
    LAST_RESULT = res

    y = np.empty((B, C, HW, HW), dtype=np.float32)
    for core in range(N_CORES):
        b, half = core // 2, core % 2
        yc = res.results[core]["y"]
        for t in range(2):
            y[b, perm[t], half * 28:(half + 1) * 28, :] = yc[:, t]
    return y


# revision 5
# speedup vs baseline: 1.0212x; 1.0212x over previous
"""Involution kernel for Trainium2, 8 NeuronCores (v3).

Sharding: data-parallel over (batch=4) x (H halves=2); 28 output rows
per core, 3-row halo, 60-wide padded raster (W=60: all tap shifts stay
8B-aligned since 60 % 4 == 0; wrapped +-3-col tap reads land in the
2-col side pads of adjacent rows, which are zero == conv padding).

Channels are cg-major on partitions (partition p of tile t <-> channel
(p%16)*16 + p//16 + 8t), so the 16 groups sit on p%16 and one 8x
row-replicated span matmul yields per-pixel weights wm_k [128, span]
valid for BOTH channel tiles.

Per tap k:
  PE:   wm_k = w2rep_k^T @ r   (2-bank half-tap PSUM tiles, dbl-buffered)
  Act:  wmb_k = wm_k + b2_k -> fp16 SBUF, compact 28x56 (b2 via bias)
  DVE:  prod = wmb_k * x_shift  [128, 2, 28, 56] both tiles, 2x mode
  PE:   acc_t0 += prod[:,0]     (identity matmul into 4-bank PSUM acc)
  SWDGE/DVE: acc_t1 += prod[:,1] (gpsimd accumulate-DMA or DVE add)
Outputs are emitted fp16 (host converts to f32): acc_t0 PSUM -> ScalarE
fp16 copy -> DMA; acc_t1 fp16 -> plain DMA. Phase-A relu runs on DVE
(idle during phase A) as tensor_scalar (add b1, max 0).
"""

import os
import sys
import numpy as np

for _p in ("/opt/trn_rl_repo",):
    if _p not in sys.path:
        sys.path.insert(0, _p)

import concourse.bass as bass
import concourse.tile as tile
from concourse import mybir
from concourse.bass_utils import run_bass_kernel_spmd
import bass_rust

F32 = mybir.dt.float32
F16 = mybir.dt.float16

N_CORES = 8
C = 256
RED = 128
K = 7
K2 = 49
GC = 16
HW = 56
W60 = 60
ROWS = 34
NPIX = ROWS * W60        # 2040
PAD = 8
XLEN = NPIX + 2 * PAD    # 2056
Q0 = 3 * W60             # 180
SPAN = 28 * W60          # 1680 raster cols covering own rows
CH = SPAN // 4           # 420 raster cols per PSUM bank
NOUT = 28 * HW           # 1568 compact output pixels
CHO = NOUT // 4          # 392 compact cols per PSUM bank
OUT0 = PAD + Q0 + 2      # flat index of first own output pixel

def _tap_delta(k):
    di, dj = k // K, k % K
    return (di - 3) * W60 + (dj - 3)


# process taps grouped by which shifted x copy they read: j==0 taps first
# (xs0 is the first copy DMA'd in), then j==1, 2, 3 as the copies stream
# in; within a group, halo-free taps (di==3) first so the very first taps
# don't wait on the halo-row loads
TAPORD = sorted(range(K2), key=lambda k: (_tap_delta(k) % 4, k // K != 3, k))
# positions (in TAPORD) whose t1-accumulate runs on DVE instead of the
# gpsimd accum-DMA; the tail block keeps its latency off the SWDGE path
DVE_ACC_POS = frozenset({45, 46, 47, 48})
W2CHUNKS = 4


def _split_multi_waits(nc, maxw=1):
    """This walrus build caps sync-wait commands per instruction; move
    excess waits onto same-engine nops inserted immediately before."""
    ctr = 0
    for fn in nc.m.functions:
        for bb in fn.blocks:
            insts = bb.instructions  # live list
            i = 0
            while i < len(insts):
                ins = insts[i]
                si = ins.sync_info
                waits = list(si.on_wait) if si is not None else []
                if len(waits) > maxw:
                    excess, keep = waits[:-maxw], waits[-maxw:]
                    for j in range(0, len(excess), maxw):
                        ctr += 1
                        nop = mybir.InstNoOp(
                            name=f"waitsplit-{ctr}",
                            engine=ins.engine,
                            bass_nofuse=True,
                            sync_info=mybir.SyncInfo(
                                on_wait=excess[j:j + maxw], on_update=[]
                            ),
                        )
                        insts.insert(i, nop)
                        i += 1
                    ins.sync_info = bass_rust.SyncInfo(
                        on_wait=keep, on_update=list(si.on_update)
                    )
                i += 1


def build_program():
    nc = bass.Bass("TRN2", target_bir_lowering=False, num_devices=N_CORES)

    xs_d = [nc.dram_tensor(f"xs{j}", [128, 2 * XLEN], F16, kind="ExternalInput")
            for j in range(4)]
    # packed small weights in one fp16 DMA (w1 tiles | identity | b1 | b2)
    # -- each HWDGE DMA costs ~625ns serialized at startup
    p16_d = nc.dram_tensor("p16", [128, 3 * RED + 2 + K2 + 1], F16,
                           kind="ExternalInput")
    # w2 blocks stored in TAPORD processing order, split into chunks
    w2_d = nc.dram_tensor("w2L", [RED, K2 * 128], F16, kind="ExternalInput")
    y_d = nc.dram_tensor("y", [128, 2, 28, HW], F16, kind="ExternalOutput")
    csz = [(K2 + W2CHUNKS - 1) // W2CHUNKS] * W2CHUNKS
    csz[-1] = K2 - sum(csz[:-1])

    with tile.TileContext(nc) as tc:
        with (
            tc.tile_pool(name="sb", bufs=1) as sb,
            tc.tile_pool(name="dbl", bufs=6) as dbl,
            tc.tile_pool(name="ps", bufs=2, space="PSUM") as ps,
            tc.tile_pool(name="psacc", bufs=1, space="PSUM") as psacc,
        ):
            # load order: packed small weights, xs0 own rows (gates phase
            # A), first tap's w2 block, halos, then the rest streaming in
            p16 = sb.tile([128, 3 * RED + 2 + K2 + 1], F16, tag="p16")
            nc.sync.dma_start(out=p16[:], in_=p16_d[:, :])
            w1sb = [p16[:, 0:RED], p16[:, RED:2 * RED]]
            idsb = p16[:, 2 * RED:3 * RED]
            # b1 stored as f32 bits in two fp16 slots (tensor_scalar
            # requires an f32 scalar for add)
            b1sb = p16[:, 3 * RED:3 * RED + 2].bitcast(F32)
            b2sb = p16[:, 3 * RED + 2:]

            xs = []
            for j in range(4):
                t = sb.tile([128, 2 * XLEN], F16, tag=f"xs{j}")
                xs.append(t)
            xs0v = xs[0][:].rearrange("p (t l) -> p t l", t=2)
            xs0dv = xs_d[0][:, :].rearrange("p (t l) -> p t l", t=2)
            for h in range(2):
                o = PAD + Q0 + h * (SPAN // 2)
                nc.sync.dma_start(
                    out=xs0v[:, :, o:o + SPAN // 2],
                    in_=xs0dv[:, :, o:o + SPAN // 2],
                )
            w2sb = sb.tile([RED, K2 * 128], F16, tag="w2")
            nc.sync.dma_start(out=w2sb[:, 0:128], in_=w2_d[:, 0:128])
            nc.sync.dma_start(
                out=xs0v[:, :, 0:PAD + Q0],
                in_=xs0dv[:, :, 0:PAD + Q0],
            )
            nc.sync.dma_start(
                out=xs0v[:, :, PAD + Q0 + SPAN:XLEN],
                in_=xs0dv[:, :, PAD + Q0 + SPAN:XLEN],
            )

            co = 1
            for i, csz_i in enumerate(csz):
                csz_i = min(csz_i, K2 - co)
                nc.sync.dma_start(
                    out=w2sb[:, co * 128:(co + csz_i) * 128],
                    in_=w2_d[:, co * 128:(co + csz_i) * 128],
                )
                co += csz_i
                if i == 0:
                    nc.sync.dma_start(out=xs[1][:], in_=xs_d[1][:, :])
                elif i == 1:
                    nc.sync.dma_start(out=xs[2][:], in_=xs_d[2][:, :])
                elif i == 2:
                    nc.sync.dma_start(out=xs[3][:], in_=xs_d[3][:, :])

            # Phase A: r = relu(w1' @ x + b1')  [128, SPAN] fp16, 60-raster
            # (per-chunk relu so the first span matmuls release early)
            r_sb = sb.tile([RED, SPAN], F16, tag="r")
            for h in range(2):
                rps = ps.tile([128, 2, 512], F32, tag="wmps")
                for cc in range(2):
                    c4 = h * 2 + cc
                    for ci in range(2):
                        nc.tensor.matmul(
                            rps[:, cc, 0:CH],
                            w1sb[ci],
                            xs[0][:, ci * XLEN + PAD + Q0 + c4 * CH:
                                  ci * XLEN + PAD + Q0 + (c4 + 1) * CH],
                            start=(ci == 0),
                            stop=(ci == 1),
                        )
                    nc.vector.tensor_scalar(
                        out=r_sb[:, c4 * CH:(c4 + 1) * CH],
                        in0=rps[:, cc, 0:CH],
                        scalar1=b1sb,
                        scalar2=0.0,
                        op0=mybir.AluOpType.add,
                        op1=mybir.AluOpType.max,
                    )

            # Phase B
            acc_ps = psacc.tile([128, 4, 512], F32, tag="acc")
            # 4 independent t1 partial accumulators -> 4 parallel RMW chains
            NACC = 4
            acc1s = []
            for i in range(NACC):
                a1t = sb.tile([128, NOUT], F16, tag=f"acc1_{i}")
                acc1s.append(a1t)
            acc1_first = [True] * NACC
            for pos, k in enumerate(TAPORD):
                delta = _tap_delta(k)
                j = delta % 4
                b0 = OUT0 + delta - j

                prod = dbl.tile([128, 2, NOUT], F16, tag="prod")
                prod_v = prod[:].rearrange("p t (hr c) -> p t hr c", c=HW)
                wmb = dbl.tile([128, NOUT], F16, tag="wmb")
                wmb_v = wmb[:].rearrange("p (h b r c) -> p h b r c",
                                         h=2, b=2, r=7, c=HW)
                wmb_m = wmb[:].rearrange("p (h rr c) -> p h rr c",
                                         h=2, rr=14, c=HW)
                xin = (xs[j][:].rearrange("p (t l) -> p t l", t=2)
                       [:, :, b0:b0 + 28 * W60]
                       .rearrange("p t (h r c) -> p t h r c", h=2, c=W60)
                       [:, :, :, :, 0:HW])

                # both span halves first, then both evacs(+b2): ScalarE
                # runs back-to-back instead of idling on span h1
                wmps_h = []
                for h in range(2):
                    wmps = ps.tile([128, 2, 512], F32, tag="wmps")
                    for hh in range(2):
                        c4 = h * 2 + hh
                        nc.tensor.matmul(
                            wmps[:, hh, 0:CH],
                            w2sb[:, pos * 128:(pos + 1) * 128],
                            r_sb[:, c4 * CH:(c4 + 1) * CH],
                            start=True,
                            stop=True,
                        )
                    wmps_h.append(wmps)
                for h in range(2):
                    src = wmps_h[h][:, :, 0:CH].rearrange(
                        "p b (r c) -> p b r c", c=W60)[:, :, :, 2:2 + HW]
                    nc.scalar.activation(
                        out=wmb_v[:, h],
                        in_=src,
                        func=mybir.ActivationFunctionType.Identity,
                        bias=b2sb[:, pos:pos + 1],
                        scale=1.0,
                    )

                # both-tile fused multiply on DVE (2x mode); the first taps
                # run as half multiplies so the pipeline fills faster
                wmb_b = (wmb[:].rearrange("p (r c) -> p r c", c=HW)
                         .unsqueeze(1).broadcast_to((128, 2, 28, HW)))
                xin_f = xin.rearrange("p t h r c -> p t (h r) c")
                if pos < 4:
                    for h in range(2):
                        nc.vector.tensor_tensor(
                            out=prod_v[:, :, h * 14:(h + 1) * 14],
                            in0=wmb_b[:, :, h * 14:(h + 1) * 14],
                            in1=xin_f[:, :, h * 14:(h + 1) * 14],
                            op=mybir.AluOpType.mult,
                        )
                else:
                    nc.vector.tensor_tensor(
                        out=prod_v, in0=wmb_b, in1=xin_f,
                        op=mybir.AluOpType.mult,
                    )

                # t0 accumulate on PE (identity matmul into PSUM)
                for c4 in range(4):
                    nc.tensor.matmul(
                        acc_ps[:, c4, 0:CHO],
                        idsb,
                        prod[:, 0, c4 * CHO:(c4 + 1) * CHO],
                        start=(pos == 0),
                        stop=(pos == K2 - 1),
                        skip_group_check=True,
                    )

                # t1 accumulate: gpsimd accum-DMA or DVE add; partials are
                # block-assigned (2-way interleaved per half) so early
                # partials can merge while later taps still run
                if pos >= 45:
                    ai = 0  # tail taps: short DVE chain on the idle partial 0
                else:
                    ai = (2 * ((2 * pos) // K2)) + (pos % 2)
                a1 = acc1s[ai]
                if acc1_first[ai]:
                    acc1_first[ai] = False
                    nc.sync.dma_start(out=a1[:], in_=prod[:, 1, :])
                elif pos in DVE_ACC_POS:
                    nc.vector.tensor_tensor(
                        out=a1[:], in0=prod[:, 1, :], in1=a1[:],
                        op=mybir.AluOpType.add,
                    )
                else:
                    nc.gpsimd.dma_start(
                        out=a1[:], in_=prod[:, 1, :],
                        accum_op=mybir.AluOpType.add,
                    )
                if pos == 30:
                    # partials 0/1 got their last tap at pos 24; fold 1 into 0
                    nc.vector.tensor_tensor(
                        out=acc1s[0][:], in0=acc1s[0][:], in1=acc1s[1][:],
                        op=mybir.AluOpType.add,
                    )
                elif pos == 47:
                    # a2/a3 got their last SWDGE tap at pos 44; pre-merge
                    nc.vector.tensor_tensor(
                        out=acc1s[2][:], in0=acc1s[2][:], in1=acc1s[3][:],
                        op=mybir.AluOpType.add,
                    )

            # Phase C: final partial merge and outputs, halved so the DMAs
            # overlap the merges/evacs of the other half
            y0 = dbl.tile([128, NOUT], F16, tag="y0")
            HF = NOUT // 2
            for h in range(2):
                nc.vector.tensor_tensor(
                    out=acc1s[0][:, h * HF:(h + 1) * HF],
                    in0=acc1s[0][:, h * HF:(h + 1) * HF],
                    in1=acc1s[2][:, h * HF:(h + 1) * HF],
                    op=mybir.AluOpType.add,
                )
                nc.sync.dma_start(
                    out=y_d[:, 1, h * 14:(h + 1) * 14],
                    in_=acc1s[0][:, h * HF:(h + 1) * HF]
                    .rearrange("p (r c) -> p r c", c=HW),
                )
                nc.scalar.activation(
                    out=y0[:, h * HF:(h + 1) * HF],
                    in_=acc_ps[:, 2 * h:2 * (h + 1), 0:CHO],
                    func=mybir.ActivationFunctionType.Copy,
                    bias=0.0,
                    scale=1.0,
                )
                nc.sync.dma_start(
                    out=y_d[:, 0, h * 14:(h + 1) * 14],
                    in_=y0[:, h * HF:(h + 1) * HF]
                    .rearrange("p (r c) -> p r c", c=HW),
                )
    _split_multi_waits(nc)
    return nc


_PROGRAM = None
LAST_RESULT = None


def _perm_channels(t):
    p = np.arange(128)
    return (p % 16) * GC + (p // 16) + 8 * t


def kernel(x, w1, b1, gamma, beta, run_mean, run_var, w2, b2):
    global _PROGRAM, LAST_RESULT
    x = np.asarray(x, dtype=np.float32)
    w1 = np.asarray(w1, dtype=np.float32)
    b1 = np.asarray(b1, dtype=np.float32)
    gamma = np.asarray(gamma, dtype=np.float32)
    beta = np.asarray(beta, dtype=np.float32)
    run_mean = np.asarray(run_mean, dtype=np.float32)
    run_var = np.asarray(run_var, dtype=np.float32)
    w2 = np.asarray(w2, dtype=np.float32)
    b2 = np.asarray(b2, dtype=np.float32)

    B = x.shape[0]
    s = gamma / np.sqrt(run_var + 1e-5)
    w1p = w1 * s[:, None]
    b1p = (b1 - run_mean) * s + beta

    perm = [_perm_channels(0), _perm_channels(1)]

    p16 = np.zeros((128, 3 * RED + 2 + K2 + 1), dtype=np.float16)
    for t in range(2):
        p16[:, t * RED:(t + 1) * RED] = w1p[:, perm[t]].T
    p16[:, 2 * RED:3 * RED] = np.eye(128, dtype=np.float16)
    p16[:, 3 * RED:3 * RED + 2] = (
        b1p.astype(np.float32)[:, None].view(np.float16))

    p = np.arange(128)
    w2L = np.empty((RED, K2 * 128), dtype=np.float16)
    for pos, k in enumerate(TAPORD):
        rows = (p % 16) * K2 + k
        w2L[:, pos * 128:(pos + 1) * 128] = w2[rows, :].T
        p16[:, 3 * RED + 2 + pos] = b2[rows]

    xpad = np.zeros((B, C, 62, W60), dtype=np.float32)
    xpad[:, :, 3:3 + HW, 2:2 + HW] = x

    in_maps = []
    for core in range(N_CORES):
        b, half = core // 2, core % 2
        sh = xpad[b, :, half * 28: half * 28 + ROWS, :].reshape(C, NPIX)
        xsl = np.zeros((128, 2, XLEN), dtype=np.float16)
        for t in range(2):
            xsl[:, t, PAD:PAD + NPIX] = sh[perm[t], :]
        xflat = xsl.reshape(128, 2 * XLEN)
        m = {"p16": p16, "w2L": w2L}
        for j in range(4):
            xj = np.zeros_like(xflat)
            if j == 0:
                xj[:] = xflat
            else:
                xj[:, :-j] = xflat[:, j:]
            m[f"xs{j}"] = xj
        in_maps.append(m)

    if _PROGRAM is None:
        _PROGRAM = build_program()
    trace = os.environ.get("INVOL_TRACE") == "1"
    res = run_bass_kernel_spmd(
        _PROGRAM, in_maps, list(range(N_CORES)),
        trace=trace,
        tmpdir=os.environ.get("INVOL_TRACE_DIR") or None,
    )
    LAST_RESULT = res

    y = np.empty((B, C, HW, HW), dtype=np.float32)
    for core in range(N_CORES):
        b, half = core // 2, core % 2
        yc = res.results[core]["y"].astype(np.float32)
        for t in range(2):
            y[b, perm[t], half * 28:(half + 1) * 28, :] = yc[:, t]
    return y
